# revision 35
# baseline (speedup 1.0000x reference)
"""Trainium2 Bass kernel for the LIF + linear-STDP recurrent SNN (T=64, N=2048).

v4: W-stationary, z-only state.  Under the TRN2 cost model a matmul
costs (output free size) x cycles_per_row regardless of K/M and
LDWEIGHTS is free, so every per-step product is a stationary-weights x
1-column matmul (free size 1):

  i_syn = sum_c WQ_tile(c,j)^T @ z_c            256 fp16 MMs
  b     = zhist_tile^T @ z_{t-1}                16 fp16 MMs (psum rows 64:128)
  q     = tpI^T z_{t-1}                         16 fp16 MMs (psum row 0)

Both STDP traces are linear functions of the z history, so the whole
correction collapses onto z-row coefficients:

  cvF_u = eta*0.95^{u+1} q + sum_s G[s,u] * (-eta b_s)      (2 fp32 MMs)
  corr  = sum_{u<=t-2} cvF_u z_u  (HSC tiles, 16 fp32 MMs, K = t-1)
        + cvF[t-1] z_{t-1}        (freshest row, exact: psi = ones^T
          (cv32 . CC[:,t-1]) broadcast -> scaled-identity matmul)
        + delta * tpoI            (rank-1, 16 K=1 fp32 MMs)

G is a constant antisymmetric-decay kernel; CC[:,tau] packs the cvF
coefficient column so psi needs no per-step table writes (the BIR
verifier rejects single-partition engine writes at partition offsets).
Only the z history is restriped (one SBUF->SBUF DMA per step, read
rows capped at t-2 for two steps of slack).  Spikes stream out in f16
16-column chunks so the final drain is small.

Measured: 139.1 us cost-model time (8.19x over the 1.139 ms streaming
baseline); steady step 1705 ns, decode-bound on the PE sequencer (586
instructions x 2.2 ns hwdecode + sem/drain glue); init ~29 us bound by
the serialized 8.4 MB fp16 weight load (DMA transfers serialize at
full bus rate in the cost model; the X01/const loads are routed so
step 0 finishes early and the PE pre-decodes step 1 during the W
load).  On-device: 1/131072 raster flips (an isolated spurious spike
at t=11 with no downstream cascade), rel err 3.0e-3 vs the 2e-2 gate.
"""

import numpy as np

N = 2048
T = 64
C = 16          # 128-partition chunks of the neuron dimension
P = 128
SC = 256.0      # v is carried as SC * v_reference
W_SCALE = 25.6  # = SC * DT * TAU_MEM_INV = 256 * 0.1
ETA_FOLD = 25.6e-3  # = SC * 0.1 * eta
V_TH_SC = 256.0     # threshold in scaled units

_CACHE = {}


def _build(zero_traces=True):
    """zero_traces=True drops the q (t_pre) and tpoI (t_post) terms, which
    are exactly zero when both initial traces are all-zero (as in
    setup_inputs); kernel() selects the variant from the actual inputs."""
    import concourse.mybir as mybir
    import concourse.tile as tile
    from concourse import bacc

    f32 = mybir.dt.float32
    f16 = mybir.dt.float16
    ALU = mybir.AluOpType
    ACTF = mybir.ActivationFunctionType

    nc = bacc.Bacc("TRN2", target_bir_lowering=False, debug=False, num_devices=1)
    wq_d = nc.dram_tensor("wq", [N, N], f16, kind="ExternalInput").ap()
    x_d = nc.dram_tensor("x01", [P, C * T], f32, kind="ExternalInput").ap()
    eye_d = nc.dram_tensor("eyes", [2, P, P], f32, kind="ExternalInput").ap()
    cc_d = nc.dram_tensor("ccc", [P, T + 1], f32, kind="ExternalInput").ap()
    tpre_d = nc.dram_tensor("tpre16", [P, C], f16, kind="ExternalInput").ap()
    tpost_d = nc.dram_tensor("tpost0", [P, C], f32, kind="ExternalInput").ap()
    out_d = nc.dram_tensor("zout", [P, C * T], f16, kind="ExternalOutput").ap()
    out_v = out_d.rearrange("p (c t) -> p c t", t=T)

    with tile.TileContext(nc, num_cores=1) as tc:
        with tc.tile_pool(name="persist", bufs=1) as pp, \
             tc.tile_pool(name="psc_pool", bufs=2, space="PSUM") as pscp, \
             tc.tile_pool(name="psd_pool", bufs=2, space="PSUM") as psdp:

            WQ = pp.tile([P, C, N], f16)       # WQ[p, c, i] = 25.6 * w0[i, 128c+p]
            X01 = pp.tile([P, T, C], f32)      # 25.6 * x[t, 128c+p], t-major
            EY = pp.tile([P, 2, P], f32)       # k=0: 0.9*I, k=1: I
            CCT = pp.tile([P, T + 1], f32)     # row 0 / rows 64+s: coeff tables
            ONES = pp.tile([P, P], f32)        # all-ones (psi reduction)
            TPI = pp.tile([P, C], f16)         # fp16 t_pre
            ZOUT = pp.tile([P, C, T], f16)     # spike raster, col t = z_t
            HSC = pp.tile([64, N], f32)        # row s = z_s (col order 16p+c)
            TI = pp.tile([1, N], f32)          # restriped tpoI row
            AI = pp.tile([P, 2, P], f32)       # ping-pong scaled identity
            v = pp.tile([P, C], f32)
            tpoI = pp.tile([P, C], f32)
            z322 = pp.tile([P, 2, C], f32)     # ping-pong f32 spikes (buf t%2)
            m = pp.tile([P, C], f16)
            cv32 = pp.tile([P, 1], f32)        # row 0: eta*q; rows 64:128: -eta*b
            y2 = pp.tile([P, 1], f32)          # cv32 . CC[:, t-1]
            cvF = pp.tile([64, 1], f32)        # z-row coefficients
            dsb = pp.tile([1, 1], f32)         # delta = p95 . beta
            av = pp.tile([P, 1], f32)          # broadcast cvF[t-1]

            # the W load bounds step 1, and DMA transfers serialize on the
            # bus, so it goes first; X01 (needed only by step 0) follows.
            # X01 is contiguous in (c t) order, so DMA it flat.
            # X01 config goes through the Act queue so it lands early (it
            # gates step 0, whose spikes let the PE pre-decode step 1's
            # stream during the serialized W load).
            nc.scalar.dma_start(X01[:, :, :].rearrange("p t c -> p (t c)"), x_d)
            for c in range(C):
                nc.sync.dma_start(WQ[:, c, :], wq_d[c * P:(c + 1) * P, :])
            nc.sync.dma_start(EY[:, 0, :], eye_d[0, :, :])
            nc.sync.dma_start(EY[:, 1, :], eye_d[1, :, :])
            nc.sync.dma_start(CCT[:], cc_d)
            if not zero_traces:
                nc.sync.dma_start(TPI[:], tpre_d)
                nc.sync.dma_start(tpoI[:], tpost_d)
            nc.vector.memset(v[:], 0.0)
            nc.vector.memset(ZOUT[:], 0.0)
            nc.vector.memset(HSC[:], 0.0)
            nc.vector.memset(ONES[:], 1.0)
            nc.vector.memset(cv32[:], 0.0)
            if not zero_traces:
                nc.sync.dma_start(TI[0:1, :], tpoI[:])  # restripe tpoI once

            NA = 112  # W0 matmuls issued before the cv32-dependent block

            for t in range(T):
                if t == 0:
                    nc.vector.tensor_copy(v[:], X01[:, 0, :])
                    psc_ap = v[:, :]
                else:
                    zf = z322[:, (t - 1) % 2, :]
                    # --- q + z-history dots, partition-major ---
                    psd = psdp.tile([P, 4], f32, tag="psd")
                    if not zero_traces:
                        for c in range(C):
                            nc.tensor.matmul(
                                psd[0:1, 0:1], TPI[:, c:c + 1],
                                ZOUT[:, c, t - 1:t],
                                start=(c == 0), stop=(c == C - 1),
                                skip_group_check=True)
                    for c in range(C):
                        nc.tensor.matmul(
                            psd[64:128, 0:1], ZOUT[:, c, 0:64],
                            ZOUT[:, c, t - 1:t],
                            start=(c == 0), stop=(c == C - 1),
                            skip_group_check=True)
                    # cv32 row 0 = +eta*q, rows 64:128 = -eta*b
                    if not zero_traces:
                        nc.scalar.activation(cv32[0:1, 0:1], psd[0:1, 0:1],
                                             ACTF.Copy, scale=ETA_FOLD)
                    nc.scalar.activation(cv32[64:128, 0:1], psd[64:128, 0:1],
                                         ACTF.Copy, scale=-ETA_FOLD)
                    # y2 = cv32 . CC[:, t-1]; psi = sum(y2) broadcast (below)
                    nc.vector.tensor_tensor(out=y2[:, 0:1], in0=cv32[:, 0:1],
                                            in1=CCT[:, t - 1:t], op=ALU.mult)
                    # --- main accumulation (W0 part A) ---
                    psc = pscp.tile([P, C], f32, tag="psc")
                    k = 0
                    for j in range(C):
                        for cp in range(C):
                            if k == NA:
                                # cvF = P95R^T (eta q) + G^T (-eta b)
                                if not zero_traces:
                                    nc.tensor.matmul(psd[0:64, 2:3],
                                                     CCT[0:1, 0:64],
                                                     cv32[0:1, 0:1],
                                                     start=True, stop=False,
                                                     skip_group_check=True)
                                nc.tensor.matmul(psd[0:64, 2:3],
                                                 CCT[64:128, 0:64],
                                                 cv32[64:128, 0:1],
                                                 start=zero_traces, stop=True,
                                                 skip_group_check=True)
                                if not zero_traces:
                                    # delta = p95 . beta
                                    nc.tensor.matmul(psd[0:1, 3:4],
                                                     CCT[64:128, 64:65],
                                                     cv32[64:128, 0:1],
                                                     start=True, stop=True,
                                                     skip_group_check=True)
                            if k == NA + 24:
                                # psi = sum_k cv32[k]*CC[k, t-1], broadcast
                                nc.tensor.matmul(psd[:, 1:2], ONES[:, :],
                                                 y2[:, 0:1],
                                                 start=True, stop=True,
                                                 skip_group_check=True)
                            nc.tensor.matmul(
                                psc[:, j:j + 1],
                                WQ[:, cp, j * P:(j + 1) * P],
                                ZOUT[:, cp, t - 1:t],
                                start=(k == 0), stop=False,
                                skip_group_check=True)
                            k += 1
                    # leak + drive
                    nc.tensor.matmul(psc[:, :], EY[:, 0, :], v[:, :],
                                     start=False, stop=False, skip_group_check=True)
                    nc.tensor.matmul(psc[:, :], EY[:, 1, :], X01[:, t, :],
                                     start=False, stop=False, skip_group_check=True)
                    # z-history correction, rows 0..t-2 (2-step DMA slack)
                    if t >= 2:
                        for j in range(C):
                            nc.tensor.matmul(
                                psc[:, j:j + 1], HSC[0:t - 1, j:N:C],
                                cvF[0:t - 1, 0:1],
                                start=False, stop=False, skip_group_check=True)
                    # rank-1 tpoI term
                    if not zero_traces:
                        for j in range(C):
                            nc.tensor.matmul(
                                psc[:, j:j + 1], TI[0:1, j:N:C], dsb[0:1, 0:1],
                                start=False, stop=False, skip_group_check=True)
                    # freshest term: cvF[t-1] * z_{t-1} via scaled identity
                    ai = AI[:, t % 2, :]
                    nc.tensor.matmul(psc[:, :], ai, zf,
                                     start=False, stop=True, skip_group_check=True)
                    psc_ap = psc[:, :]

                    # scalar-engine copies feeding the late matmuls
                    nc.scalar.activation(cvF[0:64, 0:1], psd[0:64, 2:3], ACTF.Copy)
                    if not zero_traces:
                        nc.scalar.activation(dsb[0:1, 0:1], psd[0:1, 3:4], ACTF.Copy)
                    nc.scalar.activation(av[:, 0:1], psd[:, 1:2], ACTF.Copy)
                    nc.scalar.activation(ai, EY[:, 1, :], ACTF.Copy,
                                         scale=av[:, 0:1])

                # --- spike threshold + reset ---
                nc.vector.tensor_scalar(ZOUT[:, :, t], psc_ap, V_TH_SC, None,
                                        ALU.is_gt)
                if t < T - 1:
                    nc.vector.tensor_scalar(m[:], psc_ap, V_TH_SC, None,
                                            ALU.is_le)
                    if t == 0:
                        nc.vector.tensor_tensor(out=v[:], in0=v[:], in1=m[:],
                                                op=ALU.mult)
                    else:
                        nc.vector.tensor_tensor(out=v[:], in0=psc_ap, in1=m[:],
                                                op=ALU.mult)
                    zb = z322[:, t % 2, :]
                    nc.vector.tensor_copy(zb, ZOUT[:, :, t])
                    if t <= 61:
                        nc.sync.dma_start(HSC[t:t + 1, :], zb)
                # stream spikes out in chunks so the final DMA is small
                if t in (15, 31, 47, 62):
                    t0 = t - 15 if t != 62 else 48
                    nc.sync.dma_start(out_v[:, :, t0:t + 1],
                                      ZOUT[:, :, t0:t + 1])
                elif t == 63:
                    nc.sync.dma_start(out_v[:, :, 63:64], ZOUT[:, :, 63:64])

    nc.compile()
    return nc


def _get_runner(zero_traces=True):
    """Build + compile once, and cache a jitted PJRT executor so repeat
    calls skip XLA/NEFF recompilation."""
    key = ("runner", zero_traces)
    if key in _CACHE:
        return _CACHE[key]
    import sys
    if "/opt/trn_rl_repo" not in sys.path:
        sys.path.insert(0, "/opt/trn_rl_repo")
    import jax
    import concourse.mybir as mybir
    from concourse import bass2jax

    nc = _build(zero_traces)
    _CACHE["nc"] = nc
    bass2jax.install_neuronx_cc_hook()

    in_names = []
    out_names = []
    out_avals = []
    zero_outs = []
    for alloc in nc.m.functions[0].allocations:
        if not isinstance(alloc, mybir.MemoryLocationSet):
            continue
        name = alloc.memorylocations[0].name
        if alloc.kind == "ExternalInput":
            if nc.partition_id_tensor is None or name != nc.partition_id_tensor.name:
                in_names.append(name)
        elif alloc.kind == "ExternalOutput":
            out_names.append(name)
            shape = tuple(alloc.tensor_shape)
            dtype = mybir.dt.np(alloc.dtype)
            out_avals.append(jax.core.ShapedArray(shape, dtype))
            zero_outs.append(np.zeros(shape, dtype))
    n_params = len(in_names)
    all_names = in_names + out_names
    if nc.partition_id_tensor is not None:
        all_names.append(nc.partition_id_tensor.name)
    donate = tuple(range(n_params, n_params + len(out_names)))

    def _body(*args):
        operands = list(args)
        if nc.partition_id_tensor is not None:
            operands.append(bass2jax.partition_id_tensor())
        outs = bass2jax._bass_exec_p.bind(
            *operands,
            out_avals=tuple(out_avals),
            in_names=tuple(all_names),
            out_names=tuple(out_names),
            lowering_input_output_aliases=(),
            sim_require_finite=True,
            sim_require_nnan=True,
            nc=nc,
        )
        return tuple(outs)

    jitted = jax.jit(_body, donate_argnums=donate, keep_unused=True)

    def run(in_map):
        args = [np.asarray(in_map[name]) for name in in_names]
        last_err = None
        for attempt in range(3):
            try:
                outs = jitted(*args, *[z.copy() for z in zero_outs])
                return {name: np.asarray(outs[i]) for i, name in enumerate(out_names)}
            except Exception as e:  # transient NRT/device errors: retry
                last_err = e
        raise last_err

    _CACHE[key] = run
    return run


def _cc_const():
    f = np.float32
    cc = np.zeros((P, T + 1), np.float32)
    for u in range(T):
        cc[0, u] = f(0.95) ** (u + 1)            # P95R row (q coefficient)
    for s in range(T):
        for u in range(T):
            if s < u:
                cc[64 + s, u] = f(-0.05) * f(0.95) ** (u - s)
            elif s > u:
                cc[64 + s, u] = f(0.05) * f(0.95) ** (s - u)
        cc[64 + s, T] = f(0.95) ** (s + 1)        # delta column
    return cc


def kernel(exc_current, w, t_pre, t_post):
    zero_traces = not (np.any(t_pre) or np.any(t_post))
    run = _get_runner(zero_traces)
    wq = (W_SCALE * np.ascontiguousarray(w.T)).astype(np.float16)
    x01 = (W_SCALE * exc_current).astype(np.float32)          # [T, N]
    x01 = x01.reshape(T, C, P).transpose(2, 0, 1).reshape(P, T * C)
    x01 = np.ascontiguousarray(x01)
    eyes = np.stack([0.9 * np.eye(P, dtype=np.float32),
                     np.eye(P, dtype=np.float32)])

    tpre16 = np.ascontiguousarray(t_pre.astype(np.float16).reshape(C, P).T)
    tpost0 = np.ascontiguousarray(t_post.astype(np.float32).reshape(C, P).T)
    raw = run({"wq": wq, "x01": x01, "eyes": eyes, "ccc": _cc_const(),
               "tpre16": tpre16, "tpost0": tpost0})["zout"]    # [P, C*T] f16
    spikes = raw.astype(np.float32).reshape(P, C, T).transpose(2, 1, 0).reshape(T, N)
    return np.ascontiguousarray(spikes)


# revision 38
# speedup vs baseline: 1.0052x; 1.0052x over previous
"""Trainium2 Bass kernel for the LIF + linear-STDP recurrent SNN (T=64, N=2048).

v4: W-stationary, z-only state.  Under the TRN2 cost model a matmul
costs (output free size) x cycles_per_row regardless of K/M and
LDWEIGHTS is free, so every per-step product is a stationary-weights x
1-column matmul (free size 1):

  i_syn = sum_c WQ_tile(c,j)^T @ z_c            256 fp16 MMs
  b     = zhist_tile^T @ z_{t-1}                16 fp16 MMs (psum rows 64:128)
  q     = tpI^T z_{t-1}                         16 fp16 MMs (psum row 0)

Both STDP traces are linear functions of the z history, so the whole
correction collapses onto z-row coefficients:

  cvF_u = eta*0.95^{u+1} q + sum_s G[s,u] * (-eta b_s)      (2 fp32 MMs)
  corr  = sum_{u<=t-2} cvF_u z_u  (HSC tiles, 16 fp32 MMs, K = t-1)
        + cvF[t-1] z_{t-1}        (freshest row, exact: psi = ones^T
          (cv32 . CC[:,t-1]) broadcast -> scaled-identity matmul)
        + delta * tpoI            (rank-1, 16 K=1 fp32 MMs)

G is a constant antisymmetric-decay kernel; CC[:,tau] packs the cvF
coefficient column so psi needs no per-step table writes (the BIR
verifier rejects single-partition engine writes at partition offsets).
Only the z history is restriped (one SBUF->SBUF DMA per step, read
rows capped at t-2 for two steps of slack).  Spikes stream out in f16
16-column chunks so the final drain is small.

Measured: 139.1 us cost-model time (8.19x over the 1.139 ms streaming
baseline); steady step 1705 ns, decode-bound on the PE sequencer (586
instructions x 2.2 ns hwdecode + sem/drain glue); init ~29 us bound by
the serialized 8.4 MB fp16 weight load (DMA transfers serialize at
full bus rate in the cost model; the X01/const loads are routed so
step 0 finishes early and the PE pre-decodes step 1 during the W
load).  On-device: 1/131072 raster flips (an isolated spurious spike
at t=11 with no downstream cascade), rel err 3.0e-3 vs the 2e-2 gate.
"""

import numpy as np

N = 2048
T = 64
C = 16          # 128-partition chunks of the neuron dimension
P = 128
SC = 256.0      # v is carried as SC * v_reference
W_SCALE = 25.6  # = SC * DT * TAU_MEM_INV = 256 * 0.1
ETA_FOLD = 25.6e-3  # = SC * 0.1 * eta
V_TH_SC = 256.0     # threshold in scaled units

_CACHE = {}


def _build(zero_traces=True):
    """zero_traces=True drops the q (t_pre) and tpoI (t_post) terms, which
    are exactly zero when both initial traces are all-zero (as in
    setup_inputs); kernel() selects the variant from the actual inputs."""
    import concourse.mybir as mybir
    import concourse.tile as tile
    from concourse import bacc

    f32 = mybir.dt.float32
    f16 = mybir.dt.float16
    ALU = mybir.AluOpType
    ACTF = mybir.ActivationFunctionType

    nc = bacc.Bacc("TRN2", target_bir_lowering=False, debug=False, num_devices=1)
    wq_d = nc.dram_tensor("wq", [N, N], f16, kind="ExternalInput").ap()
    x_d = nc.dram_tensor("x01", [P, C * T], f32, kind="ExternalInput").ap()
    eye_d = nc.dram_tensor("eyes", [2, P, P], f32, kind="ExternalInput").ap()
    cc_d = nc.dram_tensor("ccc", [P, T + 1], f32, kind="ExternalInput").ap()
    tpre_d = nc.dram_tensor("tpre16", [P, C], f16, kind="ExternalInput").ap()
    tpost_d = nc.dram_tensor("tpost0", [P, C], f32, kind="ExternalInput").ap()
    out_d = nc.dram_tensor("zout", [P, C * T], f16, kind="ExternalOutput").ap()
    out_v = out_d.rearrange("p (c t) -> p c t", t=T)

    with tile.TileContext(nc, num_cores=1) as tc:
        with tc.tile_pool(name="persist", bufs=1) as pp, \
             tc.tile_pool(name="psc_pool", bufs=2, space="PSUM") as pscp, \
             tc.tile_pool(name="psd_pool", bufs=2, space="PSUM") as psdp:

            WQ = pp.tile([P, C, N], f16)       # WQ[p, c, i] = 25.6 * w0[i, 128c+p]
            X01 = pp.tile([P, T, C], f32)      # 25.6 * x[t, 128c+p], t-major
            EY = pp.tile([P, 2, P], f32)       # k=0: 0.9*I, k=1: I
            CCT = pp.tile([P, T + 1], f32)     # row 0 / rows 64+s: coeff tables
            ONES = pp.tile([P, P], f32)        # all-ones (psi reduction)
            TPI = pp.tile([P, C], f16)         # fp16 t_pre
            ZOUT = pp.tile([P, C, T], f16)     # spike raster, col t = z_t
            HSC = pp.tile([64, N], f32)        # row s = z_s (col order 16p+c)
            TI = pp.tile([1, N], f32)          # restriped tpoI row
            AI = pp.tile([P, 2, P], f32)       # ping-pong scaled identity
            v = pp.tile([P, C], f32)
            tpoI = pp.tile([P, C], f32)
            z322 = pp.tile([P, 2, C], f32)     # ping-pong f32 spikes (buf t%2)
            m = pp.tile([P, C], f16)
            cv32 = pp.tile([P, 1], f32)        # row 0: eta*q; rows 64:128: -eta*b
            y2 = pp.tile([P, 1], f32)          # cv32 . CC[:, t-1]
            cvF = pp.tile([64, 1], f32)        # z-row coefficients
            dsb = pp.tile([1, 1], f32)         # delta = p95 . beta
            av = pp.tile([P, 1], f32)          # broadcast cvF[t-1]

            # the W load bounds step 1, and DMA transfers serialize on the
            # bus, so it goes first; X01 (needed only by step 0) follows.
            # X01 is contiguous in (c t) order, so DMA it flat.
            # X01 config goes through the Act queue so it lands early (it
            # gates step 0, whose spikes let the PE pre-decode step 1's
            # stream during the serialized W load).
            nc.scalar.dma_start(X01[:, :, :].rearrange("p t c -> p (t c)"), x_d)
            for c in range(C):
                nc.sync.dma_start(WQ[:, c, :], wq_d[c * P:(c + 1) * P, :])
            nc.sync.dma_start(EY[:, 0, :], eye_d[0, :, :])
            nc.sync.dma_start(EY[:, 1, :], eye_d[1, :, :])
            nc.sync.dma_start(CCT[:], cc_d)
            if not zero_traces:
                nc.sync.dma_start(TPI[:], tpre_d)
                nc.sync.dma_start(tpoI[:], tpost_d)
            nc.vector.memset(v[:], 0.0)
            nc.vector.memset(ZOUT[:], 0.0)
            nc.vector.memset(HSC[:], 0.0)
            nc.vector.memset(ONES[:], 1.0)
            nc.vector.memset(cv32[:], 0.0)
            if not zero_traces:
                nc.sync.dma_start(TI[0:1, :], tpoI[:])  # restripe tpoI once

            NA = 112  # W0 matmuls issued before the cv32-dependent block

            for t in range(T):
                if t == 0:
                    nc.vector.tensor_copy(v[:], X01[:, 0, :])
                    psc_ap = v[:, :]
                else:
                    zf = z322[:, (t - 1) % 2, :]
                    # --- q + z-history dots, partition-major ---
                    psd = psdp.tile([P, 4], f32, tag="psd")
                    if not zero_traces:
                        for c in range(C):
                            nc.tensor.matmul(
                                psd[0:1, 0:1], TPI[:, c:c + 1],
                                ZOUT[:, c, t - 1:t],
                                start=(c == 0), stop=(c == C - 1),
                                skip_group_check=True)
                    for c in range(C):
                        nc.tensor.matmul(
                            psd[64:128, 0:1], ZOUT[:, c, 0:64],
                            ZOUT[:, c, t - 1:t],
                            start=(c == 0), stop=(c == C - 1),
                            skip_group_check=True)
                    # cv32 row 0 = +eta*q, rows 64:128 = -eta*b
                    if not zero_traces:
                        nc.scalar.activation(cv32[0:1, 0:1], psd[0:1, 0:1],
                                             ACTF.Copy, scale=ETA_FOLD)
                    nc.scalar.activation(cv32[64:128, 0:1], psd[64:128, 0:1],
                                         ACTF.Copy, scale=-ETA_FOLD)
                    # y2 = cv32 . CC[:, t-1]; psi = sum(y2) broadcast (below)
                    nc.vector.tensor_tensor(out=y2[:, 0:1], in0=cv32[:, 0:1],
                                            in1=CCT[:, t - 1:t], op=ALU.mult)
                    # --- main accumulation ---
                    # cp-major: chunk cp's matmuls sit consecutively, so at
                    # step 1 the decode pipelines with the W chunk arrivals
                    # instead of stalling on the last chunk at position 16
                    psc = pscp.tile([P, C], f32, tag="psc")
                    k = 0
                    for cp in range(C):
                        for j in range(C):
                            if k == NA:
                                # cvF = P95R^T (eta q) + G^T (-eta b)
                                if not zero_traces:
                                    nc.tensor.matmul(psd[0:64, 2:3],
                                                     CCT[0:1, 0:64],
                                                     cv32[0:1, 0:1],
                                                     start=True, stop=False,
                                                     skip_group_check=True)
                                nc.tensor.matmul(psd[0:64, 2:3],
                                                 CCT[64:128, 0:64],
                                                 cv32[64:128, 0:1],
                                                 start=zero_traces, stop=True,
                                                 skip_group_check=True)
                                if not zero_traces:
                                    # delta = p95 . beta
                                    nc.tensor.matmul(psd[0:1, 3:4],
                                                     CCT[64:128, 64:65],
                                                     cv32[64:128, 0:1],
                                                     start=True, stop=True,
                                                     skip_group_check=True)
                            if k == NA + 24:
                                # psi = sum_k cv32[k]*CC[k, t-1], broadcast
                                nc.tensor.matmul(psd[:, 1:2], ONES[:, :],
                                                 y2[:, 0:1],
                                                 start=True, stop=True,
                                                 skip_group_check=True)
                            # single start on the bank's first write: on HW,
                            # start_tensor_calc zeroes beyond the written
                            # column (per-column starts lost earlier columns)
                            nc.tensor.matmul(
                                psc[:, j:j + 1],
                                WQ[:, cp, j * P:(j + 1) * P],
                                ZOUT[:, cp, t - 1:t],
                                start=(k == 0), stop=False,
                                skip_group_check=True)
                            k += 1
                    # leak + drive
                    nc.tensor.matmul(psc[:, :], EY[:, 0, :], v[:, :],
                                     start=False, stop=False, skip_group_check=True)
                    nc.tensor.matmul(psc[:, :], EY[:, 1, :], X01[:, t, :],
                                     start=False, stop=False, skip_group_check=True)
                    # z-history correction, rows 0..t-2 (2-step DMA slack)
                    if t >= 2:
                        for j in range(C):
                            nc.tensor.matmul(
                                psc[:, j:j + 1], HSC[0:t - 1, j:N:C],
                                cvF[0:t - 1, 0:1],
                                start=False, stop=False, skip_group_check=True)
                    # rank-1 tpoI term
                    if not zero_traces:
                        for j in range(C):
                            nc.tensor.matmul(
                                psc[:, j:j + 1], TI[0:1, j:N:C], dsb[0:1, 0:1],
                                start=False, stop=False, skip_group_check=True)
                    # freshest term: cvF[t-1] * z_{t-1} via scaled identity
                    ai = AI[:, t % 2, :]
                    nc.tensor.matmul(psc[:, :], ai, zf,
                                     start=False, stop=True, skip_group_check=True)
                    psc_ap = psc[:, :]

                    # scalar-engine copies feeding the late matmuls
                    nc.scalar.activation(cvF[0:64, 0:1], psd[0:64, 2:3], ACTF.Copy)
                    if not zero_traces:
                        nc.scalar.activation(dsb[0:1, 0:1], psd[0:1, 3:4], ACTF.Copy)
                    nc.scalar.activation(av[:, 0:1], psd[:, 1:2], ACTF.Copy)
                    nc.scalar.activation(ai, EY[:, 1, :], ACTF.Copy,
                                         scale=av[:, 0:1])

                # --- spike threshold + reset ---
                nc.vector.tensor_scalar(ZOUT[:, :, t], psc_ap, V_TH_SC, None,
                                        ALU.is_gt)
                if t < T - 1:
                    nc.vector.tensor_scalar(m[:], psc_ap, V_TH_SC, None,
                                            ALU.is_le)
                    if t == 0:
                        nc.vector.tensor_tensor(out=v[:], in0=v[:], in1=m[:],
                                                op=ALU.mult)
                    else:
                        nc.vector.tensor_tensor(out=v[:], in0=psc_ap, in1=m[:],
                                                op=ALU.mult)
                    zb = z322[:, t % 2, :]
                    nc.vector.tensor_copy(zb, ZOUT[:, :, t])
                    if t <= 61:
                        nc.sync.dma_start(HSC[t:t + 1, :], zb)
                # stream spikes out in chunks so the final DMA is small
                if t in (15, 31, 47, 62):
                    t0 = t - 15 if t != 62 else 48
                    nc.sync.dma_start(out_v[:, :, t0:t + 1],
                                      ZOUT[:, :, t0:t + 1])
                elif t == 63:
                    nc.sync.dma_start(out_v[:, :, 63:64], ZOUT[:, :, 63:64])

    nc.compile()
    return nc


def _get_runner(zero_traces=True):
    """Build + compile once, and cache a jitted PJRT executor so repeat
    calls skip XLA/NEFF recompilation."""
    key = ("runner", zero_traces)
    if key in _CACHE:
        return _CACHE[key]
    import sys
    if "/opt/trn_rl_repo" not in sys.path:
        sys.path.insert(0, "/opt/trn_rl_repo")
    import jax
    import concourse.mybir as mybir
    from concourse import bass2jax

    nc = _build(zero_traces)
    _CACHE["nc"] = nc
    bass2jax.install_neuronx_cc_hook()

    in_names = []
    out_names = []
    out_avals = []
    zero_outs = []
    for alloc in nc.m.functions[0].allocations:
        if not isinstance(alloc, mybir.MemoryLocationSet):
            continue
        name = alloc.memorylocations[0].name
        if alloc.kind == "ExternalInput":
            if nc.partition_id_tensor is None or name != nc.partition_id_tensor.name:
                in_names.append(name)
        elif alloc.kind == "ExternalOutput":
            out_names.append(name)
            shape = tuple(alloc.tensor_shape)
            dtype = mybir.dt.np(alloc.dtype)
            out_avals.append(jax.core.ShapedArray(shape, dtype))
            zero_outs.append(np.zeros(shape, dtype))
    n_params = len(in_names)
    all_names = in_names + out_names
    if nc.partition_id_tensor is not None:
        all_names.append(nc.partition_id_tensor.name)
    donate = tuple(range(n_params, n_params + len(out_names)))

    def _body(*args):
        operands = list(args)
        if nc.partition_id_tensor is not None:
            operands.append(bass2jax.partition_id_tensor())
        outs = bass2jax._bass_exec_p.bind(
            *operands,
            out_avals=tuple(out_avals),
            in_names=tuple(all_names),
            out_names=tuple(out_names),
            lowering_input_output_aliases=(),
            sim_require_finite=True,
            sim_require_nnan=True,
            nc=nc,
        )
        return tuple(outs)

    jitted = jax.jit(_body, donate_argnums=donate, keep_unused=True)

    def run(in_map):
        args = [np.asarray(in_map[name]) for name in in_names]
        last_err = None
        for attempt in range(3):
            try:
                outs = jitted(*args, *[z.copy() for z in zero_outs])
                return {name: np.asarray(outs[i]) for i, name in enumerate(out_names)}
            except Exception as e:  # transient NRT/device errors: retry
                last_err = e
        raise last_err

    _CACHE[key] = run
    return run


def _cc_const():
    f = np.float32
    cc = np.zeros((P, T + 1), np.float32)
    for u in range(T):
        cc[0, u] = f(0.95) ** (u + 1)            # P95R row (q coefficient)
    for s in range(T):
        for u in range(T):
            if s < u:
                cc[64 + s, u] = f(-0.05) * f(0.95) ** (u - s)
            elif s > u:
                cc[64 + s, u] = f(0.05) * f(0.95) ** (s - u)
        cc[64 + s, T] = f(0.95) ** (s + 1)        # delta column
    return cc


def kernel(exc_current, w, t_pre, t_post):
    zero_traces = not (np.any(t_pre) or np.any(t_post))
    run = _get_runner(zero_traces)
    wq = (W_SCALE * np.ascontiguousarray(w.T)).astype(np.float16)
    x01 = (W_SCALE * exc_current).astype(np.float32)          # [T, N]
    x01 = x01.reshape(T, C, P).transpose(2, 0, 1).reshape(P, T * C)
    x01 = np.ascontiguousarray(x01)
    eyes = np.stack([0.9 * np.eye(P, dtype=np.float32),
                     np.eye(P, dtype=np.float32)])

    tpre16 = np.ascontiguousarray(t_pre.astype(np.float16).reshape(C, P).T)
    tpost0 = np.ascontiguousarray(t_post.astype(np.float32).reshape(C, P).T)
    raw = run({"wq": wq, "x01": x01, "eyes": eyes, "ccc": _cc_const(),
               "tpre16": tpre16, "tpost0": tpost0})["zout"]    # [P, C*T] f16
    spikes = raw.astype(np.float32).reshape(P, C, T).transpose(2, 1, 0).reshape(T, N)
    return np.ascontiguousarray(spikes)


# revision 42
# speedup vs baseline: 1.0160x; 1.0107x over previous
"""Trainium2 Bass kernel for the LIF + linear-STDP recurrent SNN (T=64, N=2048).

v4: W-stationary, z-only state.  Under the TRN2 cost model a matmul
costs (output free size) x cycles_per_row regardless of K/M and
LDWEIGHTS is free, so every per-step product is a stationary-weights x
1-column matmul (free size 1):

  i_syn = sum_c WQ_tile(c,j)^T @ z_c            256 fp16 MMs
  b     = zhist_tile^T @ z_{t-1}                16 fp16 MMs (psum rows 64:128)
  q     = tpI^T z_{t-1}                         16 fp16 MMs (psum row 0)

Both STDP traces are linear functions of the z history, so the whole
correction collapses onto z-row coefficients:

  cvF_u = eta*0.95^{u+1} q + sum_s G[s,u] * (-eta b_s)      (2 fp32 MMs)
  corr  = sum_{u<=t-2} cvF_u z_u  (HSC tiles, 16 fp32 MMs, K = t-1)
        + cvF[t-1] z_{t-1}        (freshest row, exact: psi = ones^T
          (cv32 . CC[:,t-1]) broadcast -> scaled-identity matmul)
        + delta * tpoI            (rank-1, 16 K=1 fp32 MMs)

G is a constant antisymmetric-decay kernel; CC[:,tau] packs the cvF
coefficient column so psi needs no per-step table writes (the BIR
verifier rejects single-partition engine writes at partition offsets).
Only the z history is restriped (one SBUF->SBUF DMA per step, read
rows capped at t-2 for two steps of slack).  Spikes stream out in f16
16-column chunks so the final drain is small.

Measured: 138.4 us cost-model time (8.23x over the 1.139 ms streaming
baseline); steady step 1705 ns, decode-bound on the PE sequencer (586
instructions x 2.2 ns hwdecode + sem/drain glue); init ~29 us bound by
the serialized 8.4 MB fp16 weight load (DMA transfers serialize at
full bus rate in the cost model; the X01/const loads are routed so
step 0 finishes early, and the W0 stream is chunk-major so step 1's
decode pipelines with the chunk arrivals).  On-device: 1/131072
raster flips (an isolated spurious spike at t=11 with no downstream
cascade), rel err 3.0e-3 vs the 2e-2 gate.
"""

import numpy as np

N = 2048
T = 64
C = 16          # 128-partition chunks of the neuron dimension
P = 128
SC = 256.0      # v is carried as SC * v_reference
W_SCALE = 25.6  # = SC * DT * TAU_MEM_INV = 256 * 0.1
ETA_FOLD = 25.6e-3  # = SC * 0.1 * eta
V_TH_SC = 256.0     # threshold in scaled units

_CACHE = {}


def _build(zero_traces=True):
    """zero_traces=True drops the q (t_pre) and tpoI (t_post) terms, which
    are exactly zero when both initial traces are all-zero (as in
    setup_inputs); kernel() selects the variant from the actual inputs."""
    import concourse.mybir as mybir
    import concourse.tile as tile
    from concourse import bacc

    f32 = mybir.dt.float32
    f16 = mybir.dt.float16
    ALU = mybir.AluOpType
    ACTF = mybir.ActivationFunctionType

    nc = bacc.Bacc("TRN2", target_bir_lowering=False, debug=False, num_devices=1)
    wq_d = nc.dram_tensor("wq", [N, N], f16, kind="ExternalInput").ap()
    x_d = nc.dram_tensor("x01", [P, C * T], f32, kind="ExternalInput").ap()
    eye_d = nc.dram_tensor("eyes", [2, P, P], f32, kind="ExternalInput").ap()
    cc_d = nc.dram_tensor("ccc", [P, T + 1], f32, kind="ExternalInput").ap()
    tpre_d = nc.dram_tensor("tpre16", [P, C], f16, kind="ExternalInput").ap()
    tpost_d = nc.dram_tensor("tpost0", [P, C], f32, kind="ExternalInput").ap()
    out_d = nc.dram_tensor("zout", [P, C * T], f16, kind="ExternalOutput").ap()
    out_v = out_d.rearrange("p (c t) -> p c t", t=T)

    with tile.TileContext(nc, num_cores=1) as tc:
        with tc.tile_pool(name="persist", bufs=1) as pp, \
             tc.tile_pool(name="psc_pool", bufs=2, space="PSUM") as pscp, \
             tc.tile_pool(name="psd_pool", bufs=2, space="PSUM") as psdp:

            WQ = pp.tile([P, C, N], f16)       # WQ[p, c, i] = 25.6 * w0[i, 128c+p]
            X01 = pp.tile([P, T, C], f32)      # 25.6 * x[t, 128c+p], t-major
            EY = pp.tile([P, 2, P], f32)       # k=0: 0.9*I, k=1: I
            CCT = pp.tile([P, T + 1], f32)     # row 0 / rows 64+s: coeff tables
            ONES = pp.tile([P, P], f32)        # all-ones (psi reduction)
            TPI = pp.tile([P, C], f16)         # fp16 t_pre
            ZOUT = pp.tile([P, C, T], f16)     # spike raster, col t = z_t
            HSC = pp.tile([64, N], f32)        # row s = z_s (col order 16p+c)
            TI = pp.tile([1, N], f32)          # restriped tpoI row
            AI = pp.tile([P, 2, P], f32)       # ping-pong scaled identity
            v = pp.tile([P, C], f32)
            v2 = pp.tile([P, C], f32)          # 0.9*v + x_{t+1}
            tpoI = pp.tile([P, C], f32)
            z322 = pp.tile([P, 2, C], f32)     # ping-pong f32 spikes (buf t%2)
            m = pp.tile([P, C], f16)
            cv32 = pp.tile([P, 1], f32)        # row 0: eta*q; rows 64:128: -eta*b
            y2 = pp.tile([P, 1], f32)          # cv32 . CC[:, t-1]
            cvF = pp.tile([64, 1], f32)        # z-row coefficients
            dsb = pp.tile([1, 1], f32)         # delta = p95 . beta
            av = pp.tile([P, 1], f32)          # broadcast cvF[t-1]

            # the W load bounds step 1, and DMA transfers serialize on the
            # bus, so it goes first; X01 (needed only by step 0) follows.
            # X01 is contiguous in (c t) order, so DMA it flat.
            # X01 config goes through the Act queue so it lands early (it
            # gates step 0, whose spikes let the PE pre-decode step 1's
            # stream during the serialized W load).
            nc.scalar.dma_start(X01[:, :, :].rearrange("p t c -> p (t c)"), x_d)
            for c in range(C):
                nc.sync.dma_start(WQ[:, c, :], wq_d[c * P:(c + 1) * P, :])
            nc.sync.dma_start(EY[:, 0, :], eye_d[0, :, :])
            nc.sync.dma_start(EY[:, 1, :], eye_d[1, :, :])
            nc.sync.dma_start(CCT[:], cc_d)
            if not zero_traces:
                nc.sync.dma_start(TPI[:], tpre_d)
                nc.sync.dma_start(tpoI[:], tpost_d)
            nc.vector.memset(v[:], 0.0)
            nc.vector.memset(ZOUT[:], 0.0)
            nc.vector.memset(HSC[:], 0.0)
            nc.vector.memset(ONES[:], 1.0)
            nc.vector.memset(cv32[:], 0.0)
            if not zero_traces:
                nc.sync.dma_start(TI[0:1, :], tpoI[:])  # restripe tpoI once

            NA = 112  # W0 matmuls issued before the cv32-dependent block

            for t in range(T):
                if t == 0:
                    nc.vector.tensor_copy(v[:], X01[:, 0, :])
                    psc_ap = v[:, :]
                else:
                    zf = z322[:, (t - 1) % 2, :]
                    # --- q + z-history dots, partition-major ---
                    psd = psdp.tile([P, 4], f32, tag="psd")
                    if not zero_traces:
                        for c in range(C):
                            nc.tensor.matmul(
                                psd[0:1, 0:1], TPI[:, c:c + 1],
                                ZOUT[:, c, t - 1:t],
                                start=(c == 0), stop=(c == C - 1),
                                skip_group_check=True)
                    for c in range(C):
                        nc.tensor.matmul(
                            psd[64:128, 0:1], ZOUT[:, c, 0:64],
                            ZOUT[:, c, t - 1:t],
                            start=(c == 0), stop=(c == C - 1),
                            skip_group_check=True)
                    # cv32 row 0 = +eta*q, rows 64:128 = -eta*b
                    if not zero_traces:
                        nc.scalar.activation(cv32[0:1, 0:1], psd[0:1, 0:1],
                                             ACTF.Copy, scale=ETA_FOLD)
                    nc.scalar.activation(cv32[64:128, 0:1], psd[64:128, 0:1],
                                         ACTF.Copy, scale=-ETA_FOLD)
                    # y2 = cv32 . CC[:, t-1]; psi = sum(y2) broadcast (below)
                    nc.vector.tensor_tensor(out=y2[:, 0:1], in0=cv32[:, 0:1],
                                            in1=CCT[:, t - 1:t], op=ALU.mult)
                    # --- main accumulation ---
                    # cp-major: chunk cp's matmuls sit consecutively, so at
                    # step 1 the decode pipelines with the W chunk arrivals
                    # instead of stalling on the last chunk at position 16
                    psc = pscp.tile([P, C], f32, tag="psc")
                    k = 0
                    for cp in range(C):
                        for j in range(C):
                            if k == NA:
                                # cvF = P95R^T (eta q) + G^T (-eta b)
                                if not zero_traces:
                                    nc.tensor.matmul(psd[0:64, 2:3],
                                                     CCT[0:1, 0:64],
                                                     cv32[0:1, 0:1],
                                                     start=True, stop=False,
                                                     skip_group_check=True)
                                nc.tensor.matmul(psd[0:64, 2:3],
                                                 CCT[64:128, 0:64],
                                                 cv32[64:128, 0:1],
                                                 start=zero_traces, stop=True,
                                                 skip_group_check=True)
                                if not zero_traces:
                                    # delta = p95 . beta
                                    nc.tensor.matmul(psd[0:1, 3:4],
                                                     CCT[64:128, 64:65],
                                                     cv32[64:128, 0:1],
                                                     start=True, stop=True,
                                                     skip_group_check=True)
                            if k == NA + 24:
                                # psi = sum_k cv32[k]*CC[k, t-1], broadcast
                                nc.tensor.matmul(psd[:, 1:2], ONES[:, :],
                                                 y2[:, 0:1],
                                                 start=True, stop=True,
                                                 skip_group_check=True)
                            # single start on the bank's first write: on HW,
                            # start_tensor_calc zeroes beyond the written
                            # column (per-column starts lost earlier columns)
                            nc.tensor.matmul(
                                psc[:, j:j + 1],
                                WQ[:, cp, j * P:(j + 1) * P],
                                ZOUT[:, cp, t - 1:t],
                                start=(k == 0), stop=False,
                                skip_group_check=True)
                            k += 1
                    # leak + drive: v2 = 0.9*v + x_t precomputed on DVE in
                    # the previous step's tail, injected with one matmul
                    nc.tensor.matmul(psc[:, :], EY[:, 1, :], v2[:, :],
                                     start=False, stop=False, skip_group_check=True)
                    # z-history correction, rows 0..t-2 (2-step DMA slack)
                    if t >= 2:
                        for j in range(C):
                            nc.tensor.matmul(
                                psc[:, j:j + 1], HSC[0:t - 1, j:N:C],
                                cvF[0:t - 1, 0:1],
                                start=False, stop=False, skip_group_check=True)
                    # rank-1 tpoI term
                    if not zero_traces:
                        for j in range(C):
                            nc.tensor.matmul(
                                psc[:, j:j + 1], TI[0:1, j:N:C], dsb[0:1, 0:1],
                                start=False, stop=False, skip_group_check=True)
                    # freshest term: cvF[t-1] * z_{t-1} via scaled identity
                    ai = AI[:, t % 2, :]
                    nc.tensor.matmul(psc[:, :], ai, zf,
                                     start=False, stop=True, skip_group_check=True)
                    psc_ap = psc[:, :]

                    # scalar-engine copies feeding the late matmuls
                    nc.scalar.activation(cvF[0:64, 0:1], psd[0:64, 2:3], ACTF.Copy)
                    if not zero_traces:
                        nc.scalar.activation(dsb[0:1, 0:1], psd[0:1, 3:4], ACTF.Copy)
                    nc.scalar.activation(av[:, 0:1], psd[:, 1:2], ACTF.Copy)
                    nc.scalar.activation(ai, EY[:, 1, :], ACTF.Copy,
                                         scale=av[:, 0:1])

                # --- spike threshold + reset ---
                nc.vector.tensor_scalar(ZOUT[:, :, t], psc_ap, V_TH_SC, None,
                                        ALU.is_gt)
                if t < T - 1:
                    nc.vector.tensor_scalar(m[:], psc_ap, V_TH_SC, None,
                                            ALU.is_le)
                    if t == 0:
                        nc.vector.tensor_tensor(out=v[:], in0=v[:], in1=m[:],
                                                op=ALU.mult)
                    else:
                        nc.vector.tensor_tensor(out=v[:], in0=psc_ap, in1=m[:],
                                                op=ALU.mult)
                    nc.vector.scalar_tensor_tensor(v2[:], v[:], 0.9,
                                                   X01[:, t + 1, :],
                                                   ALU.mult, ALU.add)
                    zb = z322[:, t % 2, :]
                    nc.vector.tensor_copy(zb, ZOUT[:, :, t])
                    if t <= 61:
                        nc.sync.dma_start(HSC[t:t + 1, :], zb)
                # stream spikes out in chunks so the final DMA is small
                if t in (15, 31, 47, 62):
                    t0 = t - 15 if t != 62 else 48
                    nc.sync.dma_start(out_v[:, :, t0:t + 1],
                                      ZOUT[:, :, t0:t + 1])
                elif t == 63:
                    nc.sync.dma_start(out_v[:, :, 63:64], ZOUT[:, :, 63:64])

    nc.compile()
    return nc


def _get_runner(zero_traces=True):
    """Build + compile once, and cache a jitted PJRT executor so repeat
    calls skip XLA/NEFF recompilation."""
    key = ("runner", zero_traces)
    if key in _CACHE:
        return _CACHE[key]
    import sys
    if "/opt/trn_rl_repo" not in sys.path:
        sys.path.insert(0, "/opt/trn_rl_repo")
    import jax
    import concourse.mybir as mybir
    from concourse import bass2jax

    nc = _build(zero_traces)
    _CACHE["nc"] = nc
    bass2jax.install_neuronx_cc_hook()

    in_names = []
    out_names = []
    out_avals = []
    zero_outs = []
    for alloc in nc.m.functions[0].allocations:
        if not isinstance(alloc, mybir.MemoryLocationSet):
            continue
        name = alloc.memorylocations[0].name
        if alloc.kind == "ExternalInput":
            if nc.partition_id_tensor is None or name != nc.partition_id_tensor.name:
                in_names.append(name)
        elif alloc.kind == "ExternalOutput":
            out_names.append(name)
            shape = tuple(alloc.tensor_shape)
            dtype = mybir.dt.np(alloc.dtype)
            out_avals.append(jax.core.ShapedArray(shape, dtype))
            zero_outs.append(np.zeros(shape, dtype))
    n_params = len(in_names)
    all_names = in_names + out_names
    if nc.partition_id_tensor is not None:
        all_names.append(nc.partition_id_tensor.name)
    donate = tuple(range(n_params, n_params + len(out_names)))

    def _body(*args):
        operands = list(args)
        if nc.partition_id_tensor is not None:
            operands.append(bass2jax.partition_id_tensor())
        outs = bass2jax._bass_exec_p.bind(
            *operands,
            out_avals=tuple(out_avals),
            in_names=tuple(all_names),
            out_names=tuple(out_names),
            lowering_input_output_aliases=(),
            sim_require_finite=True,
            sim_require_nnan=True,
            nc=nc,
        )
        return tuple(outs)

    jitted = jax.jit(_body, donate_argnums=donate, keep_unused=True)

    def run(in_map):
        args = [np.asarray(in_map[name]) for name in in_names]
        last_err = None
        for attempt in range(3):
            try:
                outs = jitted(*args, *[z.copy() for z in zero_outs])
                return {name: np.asarray(outs[i]) for i, name in enumerate(out_names)}
            except Exception as e:  # transient NRT/device errors: retry
                last_err = e
        raise last_err

    _CACHE[key] = run
    return run


def _cc_const():
    f = np.float32
    cc = np.zeros((P, T + 1), np.float32)
    for u in range(T):
        cc[0, u] = f(0.95) ** (u + 1)            # P95R row (q coefficient)
    for s in range(T):
        for u in range(T):
            if s < u:
                cc[64 + s, u] = f(-0.05) * f(0.95) ** (u - s)
            elif s > u:
                cc[64 + s, u] = f(0.05) * f(0.95) ** (s - u)
        cc[64 + s, T] = f(0.95) ** (s + 1)        # delta column
    return cc


def kernel(exc_current, w, t_pre, t_post):
    zero_traces = not (np.any(t_pre) or np.any(t_post))
    run = _get_runner(zero_traces)
    wq = (W_SCALE * np.ascontiguousarray(w.T)).astype(np.float16)
    x01 = (W_SCALE * exc_current).astype(np.float32)          # [T, N]
    x01 = x01.reshape(T, C, P).transpose(2, 0, 1).reshape(P, T * C)
    x01 = np.ascontiguousarray(x01)
    eyes = np.stack([0.9 * np.eye(P, dtype=np.float32),
                     np.eye(P, dtype=np.float32)])

    tpre16 = np.ascontiguousarray(t_pre.astype(np.float16).reshape(C, P).T)
    tpost0 = np.ascontiguousarray(t_post.astype(np.float32).reshape(C, P).T)
    raw = run({"wq": wq, "x01": x01, "eyes": eyes, "ccc": _cc_const(),
               "tpre16": tpre16, "tpost0": tpost0})["zout"]    # [P, C*T] f16
    spikes = raw.astype(np.float32).reshape(P, C, T).transpose(2, 1, 0).reshape(T, N)
    return np.ascontiguousarray(spikes)


# revision 44
# speedup vs baseline: 1.0214x; 1.0053x over previous
"""Trainium2 Bass kernel for the LIF + linear-STDP recurrent SNN (T=64, N=2048).

v4: W-stationary, z-only state.  Under the TRN2 cost model a matmul
costs (output free size) x cycles_per_row regardless of K/M and
LDWEIGHTS is free, so every per-step product is a stationary-weights x
1-column matmul (free size 1):

  i_syn = sum_c WQ_tile(c,j)^T @ z_c            256 fp16 MMs
  b     = zhist_tile^T @ z_{t-1}                16 fp16 MMs (psum rows 64:128)
  q     = tpI^T z_{t-1}                         16 fp16 MMs (psum row 0)

Both STDP traces are linear functions of the z history, so the whole
correction collapses onto z-row coefficients:

  cvF_u = eta*0.95^{u+1} q + sum_s G[s,u] * (-eta b_s)      (2 fp32 MMs)
  corr  = sum_{u<=t-2} cvF_u z_u  (HSC tiles, 16 fp32 MMs, K = t-1)
        + cvF[t-1] z_{t-1}        (freshest row, exact: psi = ones^T
          (cv32 . CC[:,t-1]) broadcast -> scaled-identity matmul)
        + delta * tpoI            (rank-1, 16 K=1 fp32 MMs)

G is a constant antisymmetric-decay kernel; CC[:,tau] packs the cvF
coefficient column so psi needs no per-step table writes (the BIR
verifier rejects single-partition engine writes at partition offsets).
Only the z history is restriped (one SBUF->SBUF DMA per step, read
rows capped at t-2 for two steps of slack).  Spikes stream out in f16
16-column chunks so the final drain is small.

Measured: 136.9 us cost-model time (8.32x over the 1.139 ms streaming
baseline); steady step ~1680 ns, decode-bound on the PE sequencer
(584 instructions x 2.2 ns hwdecode + sem/drain glue; the leak+drive
fold 0.9v+x runs on the idle DVE so only one identity matmul injects
it); init ~29 us bound by the serialized 8.4 MB fp16 weight load (DMA
transfers serialize at full bus rate in the cost model; the X01/const
loads are routed so step 0 finishes early, and the W0 stream is
chunk-major so step 1's decode pipelines with the chunk arrivals).
On-device: 1/131072 raster flips (an isolated spurious spike at t=11
with no downstream cascade), rel err 3.0e-3 vs the 2e-2 gate.
"""

import numpy as np

N = 2048
T = 64
C = 16          # 128-partition chunks of the neuron dimension
P = 128
SC = 256.0      # v is carried as SC * v_reference
W_SCALE = 25.6  # = SC * DT * TAU_MEM_INV = 256 * 0.1
ETA_FOLD = 25.6e-3  # = SC * 0.1 * eta
V_TH_SC = 256.0     # threshold in scaled units

_CACHE = {}


def _build(zero_traces=True):
    """zero_traces=True drops the q (t_pre) and tpoI (t_post) terms, which
    are exactly zero when both initial traces are all-zero (as in
    setup_inputs); kernel() selects the variant from the actual inputs."""
    import concourse.mybir as mybir
    import concourse.tile as tile
    from concourse import bacc

    f32 = mybir.dt.float32
    f16 = mybir.dt.float16
    ALU = mybir.AluOpType
    ACTF = mybir.ActivationFunctionType

    nc = bacc.Bacc("TRN2", target_bir_lowering=False, debug=False, num_devices=1)
    wq_d = nc.dram_tensor("wq", [N, N], f16, kind="ExternalInput").ap()
    x_d = nc.dram_tensor("x01", [P, C * T], f32, kind="ExternalInput").ap()
    eye_d = nc.dram_tensor("eyes", [2, P, P], f32, kind="ExternalInput").ap()
    cc_d = nc.dram_tensor("ccc", [P, T + 1], f32, kind="ExternalInput").ap()
    tpre_d = nc.dram_tensor("tpre16", [P, C], f16, kind="ExternalInput").ap()
    tpost_d = nc.dram_tensor("tpost0", [P, C], f32, kind="ExternalInput").ap()
    out_d = nc.dram_tensor("zout", [P, C * T], f16, kind="ExternalOutput").ap()
    out_v = out_d.rearrange("p (c t) -> p c t", t=T)

    with tile.TileContext(nc, num_cores=1) as tc:
        with tc.tile_pool(name="persist", bufs=1) as pp, \
             tc.tile_pool(name="psc_pool", bufs=2, space="PSUM") as pscp, \
             tc.tile_pool(name="psd_pool", bufs=2, space="PSUM") as psdp:

            WQ = pp.tile([P, C, N], f16)       # WQ[p, c, i] = 25.6 * w0[i, 128c+p]
            X01 = pp.tile([P, T, C], f32)      # 25.6 * x[t, 128c+p], t-major
            EY = pp.tile([P, 2, P], f32)       # k=0: 0.9*I, k=1: I
            CCT = pp.tile([P, T + 1], f32)     # row 0 / rows 64+s: coeff tables
            ONES = pp.tile([P, P], f32)        # all-ones (psi reduction)
            TPI = pp.tile([P, C], f16)         # fp16 t_pre
            ZOUT = pp.tile([P, C, T], f16)     # spike raster, col t = z_t
            HSC = pp.tile([64, N], f32)        # row s = z_s (col order 16p+c)
            TI = pp.tile([1, N], f32)          # restriped tpoI row
            AI = pp.tile([P, 2, P], f32)       # ping-pong scaled identity
            v = pp.tile([P, C], f32)
            v2 = pp.tile([P, C], f32)          # 0.9*v + x_{t+1}
            tpoI = pp.tile([P, C], f32)
            z322 = pp.tile([P, 2, C], f32)     # ping-pong f32 spikes (buf t%2)
            m = pp.tile([P, C], f16)
            cv32 = pp.tile([P, 1], f32)        # row 0: eta*q; rows 64:128: -eta*b
            y2 = pp.tile([P, 1], f32)          # cv32 . CC[:, t-1]
            cvF = pp.tile([64, 1], f32)        # z-row coefficients
            dsb = pp.tile([1, 1], f32)         # delta = p95 . beta
            av = pp.tile([P, 1], f32)          # broadcast cvF[t-1]

            # the W load bounds step 1, and DMA transfers serialize on the
            # bus, so it goes first; X01 (needed only by step 0) follows.
            # X01 is contiguous in (c t) order, so DMA it flat.
            # x cols 0:8 load early on SP (they gate steps 0-7, and an early
            # step 0 lets the PE pre-decode step 1 during the W load); the
            # x bulk's config sits behind the 16 W configs on the Act queue,
            # so its transfer slots in after the W load instead of extending
            # the serialized-bus critical path.
            nc.sync.dma_start(X01[:, 0:8, :].rearrange("p t c -> p (t c)"),
                              x_d[:, 0:8 * C])
            for c in range(C):
                nc.scalar.dma_start(WQ[:, c, :], wq_d[c * P:(c + 1) * P, :])
            nc.scalar.dma_start(X01[:, 8:T, :].rearrange("p t c -> p (t c)"),
                                x_d[:, 8 * C:])
            nc.sync.dma_start(EY[:, 0, :], eye_d[0, :, :])
            nc.sync.dma_start(EY[:, 1, :], eye_d[1, :, :])
            nc.sync.dma_start(CCT[:], cc_d)
            if not zero_traces:
                nc.sync.dma_start(TPI[:], tpre_d)
                nc.sync.dma_start(tpoI[:], tpost_d)
            nc.vector.memset(v[:], 0.0)
            nc.vector.memset(ZOUT[:], 0.0)
            nc.vector.memset(HSC[:], 0.0)
            nc.vector.memset(ONES[:], 1.0)
            nc.vector.memset(cv32[:], 0.0)
            if not zero_traces:
                nc.sync.dma_start(TI[0:1, :], tpoI[:])  # restripe tpoI once

            NA = 112  # W0 matmuls issued before the cv32-dependent block

            for t in range(T):
                if t == 0:
                    nc.vector.tensor_copy(v[:], X01[:, 0, :])
                    psc_ap = v[:, :]
                else:
                    zf = z322[:, (t - 1) % 2, :]
                    # --- q + z-history dots, partition-major ---
                    psd = psdp.tile([P, 4], f32, tag="psd")
                    if not zero_traces:
                        for c in range(C):
                            nc.tensor.matmul(
                                psd[0:1, 0:1], TPI[:, c:c + 1],
                                ZOUT[:, c, t - 1:t],
                                start=(c == 0), stop=(c == C - 1),
                                skip_group_check=True)
                    for c in range(C):
                        nc.tensor.matmul(
                            psd[64:128, 0:1], ZOUT[:, c, 0:64],
                            ZOUT[:, c, t - 1:t],
                            start=(c == 0), stop=(c == C - 1),
                            skip_group_check=True)
                    # cv32 row 0 = +eta*q, rows 64:128 = -eta*b
                    if not zero_traces:
                        nc.scalar.activation(cv32[0:1, 0:1], psd[0:1, 0:1],
                                             ACTF.Copy, scale=ETA_FOLD)
                    nc.scalar.activation(cv32[64:128, 0:1], psd[64:128, 0:1],
                                         ACTF.Copy, scale=-ETA_FOLD)
                    # y2 = cv32 . CC[:, t-1]; psi = sum(y2) broadcast (below)
                    nc.vector.tensor_tensor(out=y2[:, 0:1], in0=cv32[:, 0:1],
                                            in1=CCT[:, t - 1:t], op=ALU.mult)
                    # --- main accumulation ---
                    # cp-major: chunk cp's matmuls sit consecutively, so at
                    # step 1 the decode pipelines with the W chunk arrivals
                    # instead of stalling on the last chunk at position 16
                    psc = pscp.tile([P, C], f32, tag="psc")
                    k = 0
                    for cp in range(C):
                        for j in range(C):
                            if k == NA:
                                # cvF = P95R^T (eta q) + G^T (-eta b)
                                if not zero_traces:
                                    nc.tensor.matmul(psd[0:64, 2:3],
                                                     CCT[0:1, 0:64],
                                                     cv32[0:1, 0:1],
                                                     start=True, stop=False,
                                                     skip_group_check=True)
                                nc.tensor.matmul(psd[0:64, 2:3],
                                                 CCT[64:128, 0:64],
                                                 cv32[64:128, 0:1],
                                                 start=zero_traces, stop=True,
                                                 skip_group_check=True)
                                if not zero_traces:
                                    # delta = p95 . beta
                                    nc.tensor.matmul(psd[0:1, 3:4],
                                                     CCT[64:128, 64:65],
                                                     cv32[64:128, 0:1],
                                                     start=True, stop=True,
                                                     skip_group_check=True)
                            if k == NA + 24:
                                # psi = sum_k cv32[k]*CC[k, t-1], broadcast
                                nc.tensor.matmul(psd[:, 1:2], ONES[:, :],
                                                 y2[:, 0:1],
                                                 start=True, stop=True,
                                                 skip_group_check=True)
                            # single start on the bank's first write: on HW,
                            # start_tensor_calc zeroes beyond the written
                            # column (per-column starts lost earlier columns)
                            nc.tensor.matmul(
                                psc[:, j:j + 1],
                                WQ[:, cp, j * P:(j + 1) * P],
                                ZOUT[:, cp, t - 1:t],
                                start=(k == 0), stop=False,
                                skip_group_check=True)
                            k += 1
                    # leak + drive: v2 = 0.9*v + x_t precomputed on DVE in
                    # the previous step's tail, injected with one matmul
                    nc.tensor.matmul(psc[:, :], EY[:, 1, :], v2[:, :],
                                     start=False, stop=False, skip_group_check=True)
                    # z-history correction, rows 0..t-2 (2-step DMA slack)
                    if t >= 2:
                        for j in range(C):
                            nc.tensor.matmul(
                                psc[:, j:j + 1], HSC[0:t - 1, j:N:C],
                                cvF[0:t - 1, 0:1],
                                start=False, stop=False, skip_group_check=True)
                    # rank-1 tpoI term
                    if not zero_traces:
                        for j in range(C):
                            nc.tensor.matmul(
                                psc[:, j:j + 1], TI[0:1, j:N:C], dsb[0:1, 0:1],
                                start=False, stop=False, skip_group_check=True)
                    # freshest term: cvF[t-1] * z_{t-1} via scaled identity
                    ai = AI[:, t % 2, :]
                    nc.tensor.matmul(psc[:, :], ai, zf,
                                     start=False, stop=True, skip_group_check=True)
                    psc_ap = psc[:, :]

                    # scalar-engine copies feeding the late matmuls
                    nc.scalar.activation(cvF[0:64, 0:1], psd[0:64, 2:3], ACTF.Copy)
                    if not zero_traces:
                        nc.scalar.activation(dsb[0:1, 0:1], psd[0:1, 3:4], ACTF.Copy)
                    nc.scalar.activation(av[:, 0:1], psd[:, 1:2], ACTF.Copy)
                    nc.scalar.activation(ai, EY[:, 1, :], ACTF.Copy,
                                         scale=av[:, 0:1])

                # --- spike threshold + reset ---
                nc.vector.tensor_scalar(ZOUT[:, :, t], psc_ap, V_TH_SC, None,
                                        ALU.is_gt)
                if t < T - 1:
                    nc.vector.tensor_scalar(m[:], psc_ap, V_TH_SC, None,
                                            ALU.is_le)
                    if t == 0:
                        nc.vector.tensor_tensor(out=v[:], in0=v[:], in1=m[:],
                                                op=ALU.mult)
                    else:
                        nc.vector.tensor_tensor(out=v[:], in0=psc_ap, in1=m[:],
                                                op=ALU.mult)
                    nc.vector.scalar_tensor_tensor(v2[:], v[:], 0.9,
                                                   X01[:, t + 1, :],
                                                   ALU.mult, ALU.add)
                    zb = z322[:, t % 2, :]
                    nc.vector.tensor_copy(zb, ZOUT[:, :, t])
                    if t <= 61:
                        nc.sync.dma_start(HSC[t:t + 1, :], zb)
                # stream spikes out in chunks so the final DMA is small
                if t in (15, 31, 47, 62):
                    t0 = t - 15 if t != 62 else 48
                    nc.sync.dma_start(out_v[:, :, t0:t + 1],
                                      ZOUT[:, :, t0:t + 1])
                elif t == 63:
                    nc.sync.dma_start(out_v[:, :, 63:64], ZOUT[:, :, 63:64])

    nc.compile()
    return nc


def _get_runner(zero_traces=True):
    """Build + compile once, and cache a jitted PJRT executor so repeat
    calls skip XLA/NEFF recompilation."""
    key = ("runner", zero_traces)
    if key in _CACHE:
        return _CACHE[key]
    import sys
    if "/opt/trn_rl_repo" not in sys.path:
        sys.path.insert(0, "/opt/trn_rl_repo")
    import jax
    import concourse.mybir as mybir
    from concourse import bass2jax

    nc = _build(zero_traces)
    _CACHE["nc"] = nc
    bass2jax.install_neuronx_cc_hook()

    in_names = []
    out_names = []
    out_avals = []
    zero_outs = []
    for alloc in nc.m.functions[0].allocations:
        if not isinstance(alloc, mybir.MemoryLocationSet):
            continue
        name = alloc.memorylocations[0].name
        if alloc.kind == "ExternalInput":
            if nc.partition_id_tensor is None or name != nc.partition_id_tensor.name:
                in_names.append(name)
        elif alloc.kind == "ExternalOutput":
            out_names.append(name)
            shape = tuple(alloc.tensor_shape)
            dtype = mybir.dt.np(alloc.dtype)
            out_avals.append(jax.core.ShapedArray(shape, dtype))
            zero_outs.append(np.zeros(shape, dtype))
    n_params = len(in_names)
    all_names = in_names + out_names
    if nc.partition_id_tensor is not None:
        all_names.append(nc.partition_id_tensor.name)
    donate = tuple(range(n_params, n_params + len(out_names)))

    def _body(*args):
        operands = list(args)
        if nc.partition_id_tensor is not None:
            operands.append(bass2jax.partition_id_tensor())
        outs = bass2jax._bass_exec_p.bind(
            *operands,
            out_avals=tuple(out_avals),
            in_names=tuple(all_names),
            out_names=tuple(out_names),
            lowering_input_output_aliases=(),
            sim_require_finite=True,
            sim_require_nnan=True,
            nc=nc,
        )
        return tuple(outs)

    jitted = jax.jit(_body, donate_argnums=donate, keep_unused=True)

    def run(in_map):
        args = [np.asarray(in_map[name]) for name in in_names]
        last_err = None
        for attempt in range(3):
            try:
                outs = jitted(*args, *[z.copy() for z in zero_outs])
                return {name: np.asarray(outs[i]) for i, name in enumerate(out_names)}
            except Exception as e:  # transient NRT/device errors: retry
                last_err = e
        raise last_err

    _CACHE[key] = run
    return run


def _cc_const():
    f = np.float32
    cc = np.zeros((P, T + 1), np.float32)
    for u in range(T):
        cc[0, u] = f(0.95) ** (u + 1)            # P95R row (q coefficient)
    for s in range(T):
        for u in range(T):
            if s < u:
                cc[64 + s, u] = f(-0.05) * f(0.95) ** (u - s)
            elif s > u:
                cc[64 + s, u] = f(0.05) * f(0.95) ** (s - u)
        cc[64 + s, T] = f(0.95) ** (s + 1)        # delta column
    return cc


def kernel(exc_current, w, t_pre, t_post):
    zero_traces = not (np.any(t_pre) or np.any(t_post))
    run = _get_runner(zero_traces)
    wq = (W_SCALE * np.ascontiguousarray(w.T)).astype(np.float16)
    x01 = (W_SCALE * exc_current).astype(np.float32)          # [T, N]
    x01 = x01.reshape(T, C, P).transpose(2, 0, 1).reshape(P, T * C)
    x01 = np.ascontiguousarray(x01)
    eyes = np.stack([0.9 * np.eye(P, dtype=np.float32),
                     np.eye(P, dtype=np.float32)])

    tpre16 = np.ascontiguousarray(t_pre.astype(np.float16).reshape(C, P).T)
    tpost0 = np.ascontiguousarray(t_post.astype(np.float32).reshape(C, P).T)
    raw = run({"wq": wq, "x01": x01, "eyes": eyes, "ccc": _cc_const(),
               "tpre16": tpre16, "tpost0": tpost0})["zout"]    # [P, C*T] f16
    spikes = raw.astype(np.float32).reshape(P, C, T).transpose(2, 1, 0).reshape(T, N)
    return np.ascontiguousarray(spikes)
